# revision 22
# baseline (speedup 1.0000x reference)
"""Compound MoE (8 outer x 2 inner experts, top2-outer / (2,1)-inner) on 8 TRN2 cores.

Strategy (expert-parallel, per sharding hint):
  - core c owns global experts {2c, 2c+1} == both inner experts of outer expert c.
  - every core computes the (cheap) router in exact fp32 on-device, then the
    dense FFN for its two experts over all tokens in bf16 (fp32 PSUM accum),
    scales by its combine weights, and ReduceScatters the [T, D] partial sums
    (chunked by 256 tokens, overlapped with compute) so core c ends up with
    rows q*32..(q+1)*32 of each 256-token chunk q. Host reassembles.
  - routing is mask-based (no integer indices needed):
      w0 = sigmoid(l_top1 - l_top2)  (== normalized top-2 softmax weight)
      slot0 (top-1 outer o1): both inner experts of o1 get weight w0
      slot1 (top-2 outer o2): argmax inner expert of o2 gets weight w1 = 1-w0
    so this core's expert weights are
      we_even = w0*mask1[c] + w1*mask2[c]*(s0>=s1)
      we_odd  = w0*mask1[c] + w1*mask2[c]*(s1>s0)
  - chunk 0's combine is deferred (y0/y1 stashed unscaled) so the first expert
    matmuls never wait on the router, whose fp32 inputs DMA later.
"""

import numpy as np

P = 128
T = 1024
D = 1024
F = 512
NOUT = 8
KD = D // P   # 8 contraction tiles
KF = F // P   # 4 f tiles
NT = T // P   # 8 token tiles
TCH = 256     # token chunk (moving free dim for g/u matmuls)
NTCH = T // TCH          # 4
TPC = TCH // P           # token tiles per chunk = 2
NCORES = 8
RSCH = 256               # tokens per reduce-scatter chunk
NRS = T // RSCH          # 4 RS chunks
RS_ROWS = RSCH // NCORES  # 32 rows per core per RS chunk

_COMPILED = None  # cached compiled graph; compile once per process


def _build():
    import concourse.bass as bass
    import concourse.mybir as mybir
    import concourse.tile as tile
    from concourse import bacc
    from concourse.bass import ts

    fp32 = mybir.dt.float32
    bf16 = mybir.dt.bfloat16
    OP = mybir.AluOpType
    ACTF = mybir.ActivationFunctionType

    nc = bacc.Bacc(
        "TRN2",
        target_bir_lowering=False,
        debug=False,
        num_devices=NCORES,
    )

    # ---- I/O ----
    xTf = nc.dram_tensor("xTf", [D, T], fp32, kind="ExternalInput").ap()
    wg = nc.dram_tensor("wg", [2, D, F], bf16, kind="ExternalInput").ap()
    wu = nc.dram_tensor("wu", [2, D, F], bf16, kind="ExternalInput").ap()
    wd = nc.dram_tensor("wd", [2, F, D], bf16, kind="ExternalInput").ap()
    wob = nc.dram_tensor("wob", [D, 3 * NOUT], fp32, kind="ExternalInput").ap()
    selm = nc.dram_tensor("selm", [1, NOUT], fp32, kind="ExternalInput").ap()
    out_sh = nc.dram_tensor("out", [P, D], bf16, kind="ExternalOutput").ap()
    selw = nc.dram_tensor("selw", [T, 3], fp32, kind="ExternalOutput").ap()

    with tile.TileContext(nc) as tc:
        with (
            tc.tile_pool(name="res", bufs=1) as res,          # resident tensors
            tc.tile_pool(name="hbuf", bufs=2) as hbuf,        # hT double buffer
            tc.tile_pool(name="tmp", bufs=2) as tmp,          # small temporaries
            tc.tile_pool(name="psgu", bufs=4, space="PSUM") as psgu,
            tc.tile_pool(name="psy", bufs=4, space="PSUM") as psy,
            tc.tile_pool(name="dram", bufs=1, space="DRAM") as dram,
        ):
            # ---- resident SBUF tensors; DMAs split/ordered for early start ----
            # ship only fp32 x^T; derive bf16 copy on DVE as chunks land
            xTf_r = xTf.rearrange("(o p) t -> p o t", p=P)
            xTf_sb = res.tile([P, KD, T], fp32)
            xTb_sb = res.tile([P, KD, T], bf16)

            wg_sb = [res.tile([P, KD, F], bf16, tag=f"wg{s}", name=f"wg{s}") for s in range(2)]
            wu_sb = [res.tile([P, KD, F], bf16, tag=f"wu{s}", name=f"wu{s}") for s in range(2)]
            wd_sb = [res.tile([P, KF, D], bf16, tag=f"wd{s}", name=f"wd{s}") for s in range(2)]

            wgr = [wg[s].rearrange("(o p) f -> p o f", p=P) for s in range(2)]
            wur = [wu[s].rearrange("(o p) f -> p o f", p=P) for s in range(2)]
            wdr = [wd[s].rearrange("(o p) d -> p o d", p=P) for s in range(2)]

            # interleave xTf chunks (cast immediately) with expert-0 gate weights
            for kd in range(KD):
                nc.sync.dma_start(xTf_sb[:, kd, :], xTf_r[:, kd, :])
                nc.sync.dma_start(wg_sb[0][:, kd, :], wgr[0][:, kd, :])
                nc.vector.tensor_copy(xTb_sb[:, kd, :], xTf_sb[:, kd, :])
            wob_sb = res.tile([P, KD, 3 * NOUT], fp32)
            nc.sync.dma_start(wob_sb, wob.rearrange("(o p) g -> p o g", p=P))
            selm_sb = res.tile([P, NOUT], fp32)
            selm_bcast = bass.AP(
                tensor=selm.tensor, offset=selm.offset,
                ap=[[0, P], selm.ap[1]],
            )
            nc.gpsimd.dma_start(out=selm_sb, in_=selm_bcast)
            for kd in range(KD):
                nc.sync.dma_start(wu_sb[0][:, kd, :], wur[0][:, kd, :])
            for fi in range(KF):
                nc.sync.dma_start(wd_sb[0][:, fi, :], wdr[0][:, fi, :])
            for kd in range(KD):
                nc.sync.dma_start(wg_sb[1][:, kd, :], wgr[1][:, kd, :])
            for kd in range(KD):
                nc.sync.dma_start(wu_sb[1][:, kd, :], wur[1][:, kd, :])
            for fi in range(KF):
                nc.sync.dma_start(wd_sb[1][:, fi, :], wdr[1][:, fi, :])

            acc_sb = [res.tile([P, D], bf16, tag=f"acc{g}", name=f"acc{g}") for g in range(NT)]
            we0_sb = res.tile([P, NT], fp32)
            we1_sb = res.tile([P, NT], fp32)
            selw_sb = res.tile([P, NT, 3], fp32)

            # ---- router: emitted per-tile, staggered into the chunk loop ----
            def do_router(i):
                plg = psy.tile([P, 3 * NOUT], fp32, tag="y")
                for kd in range(KD):
                    nc.tensor.matmul(
                        plg,
                        lhsT=xTf_sb[:, kd, ts(i, P)],
                        rhs=wob_sb[:, kd, :],
                        start=(kd == 0),
                        stop=(kd == KD - 1),
                    )
                lg = tmp.tile([P, 3 * NOUT], fp32, tag="lg")
                nc.scalar.copy(lg, plg)

                Lg = lg[:, 0:NOUT]
                Sg = lg[:, NOUT : 3 * NOUT].rearrange("p (o i) -> p o i", i=2)

                m1 = tmp.tile([P, 1], fp32, tag="m1")
                nc.vector.tensor_reduce(m1, Lg, axis=mybir.AxisListType.X, op=OP.max)
                mask1 = tmp.tile([P, NOUT], fp32, tag="mask1")
                nc.vector.tensor_scalar(mask1, Lg, m1, None, op0=OP.is_ge)
                Lm = tmp.tile([P, NOUT], fp32, tag="Lm")
                nc.vector.scalar_tensor_tensor(
                    Lm, in0=mask1, scalar=-1.0e30, in1=Lg, op0=OP.mult, op1=OP.add
                )
                m2 = tmp.tile([P, 1], fp32, tag="m2")
                nc.vector.tensor_reduce(m2, Lm, axis=mybir.AxisListType.X, op=OP.max)
                mask2 = tmp.tile([P, NOUT], fp32, tag="mask2")
                nc.vector.tensor_scalar(mask2, Lm, m2, None, op0=OP.is_ge)

                dm = tmp.tile([P, 1], fp32, tag="dm")
                nc.vector.tensor_tensor(dm, m1, m2, op=OP.subtract)
                w0 = tmp.tile([P, 1], fp32, tag="w0")
                nc.scalar.activation(w0, dm, ACTF.Sigmoid)
                w1 = tmp.tile([P, 1], fp32, tag="w1")
                nc.scalar.activation(w1, dm, ACTF.Sigmoid, scale=-1.0)

                nc.gpsimd.tensor_copy(selw_sb[:, i, 0:2], w0.to_broadcast((P, 2)))
                nc.gpsimd.tensor_copy(selw_sb[:, i, 2:3], w1)

                dlt = tmp.tile([P, NOUT], fp32, tag="dlt")
                nc.vector.tensor_tensor(dlt, Sg[:, :, 0], Sg[:, :, 1], op=OP.subtract)
                m0 = tmp.tile([P, NOUT], fp32, tag="m0")
                nc.vector.tensor_scalar(m0, dlt, 0.0, None, op0=OP.is_ge)

                A = tmp.tile([P, NOUT], fp32, tag="A")
                nc.vector.tensor_scalar(A, mask1, w0, None, op0=OP.mult)
                Bm = tmp.tile([P, NOUT], fp32, tag="Bm")
                nc.vector.tensor_scalar(Bm, mask2, w1, None, op0=OP.mult)
                t1 = tmp.tile([P, NOUT], fp32, tag="t1")
                nc.vector.tensor_tensor(t1, Bm, m0, op=OP.mult)
                ce = tmp.tile([P, NOUT], fp32, tag="ce")
                nc.vector.tensor_tensor(ce, t1, A, op=OP.add)
                t2 = tmp.tile([P, NOUT], fp32, tag="t2")
                nc.vector.tensor_tensor(t2, Bm, t1, op=OP.subtract)
                co = tmp.tile([P, NOUT], fp32, tag="co")
                nc.vector.tensor_tensor(co, t2, A, op=OP.add)

                d0 = tmp.tile([P, NOUT], fp32, tag="d0")
                nc.vector.tensor_tensor(d0, ce, selm_sb, op=OP.mult)
                nc.vector.tensor_reduce(
                    we0_sb[:, i : i + 1], d0, axis=mybir.AxisListType.X, op=OP.add
                )
                d1 = tmp.tile([P, NOUT], fp32, tag="d1")
                nc.vector.tensor_tensor(d1, co, selm_sb, op=OP.mult)
                nc.vector.tensor_reduce(
                    we1_sb[:, i : i + 1], d1, axis=mybir.AxisListType.X, op=OP.add
                )
                nc.sync.dma_start(selw[ts(i, P), :], selw_sb[:, i, :])

            # ---- warmup collective: absorb RDH bootstrap before the real chunks ----
            warm_sb = res.tile([64, 128], bf16, name="warm_sb")
            nc.vector.memset(warm_sb, 0.0)
            warm_in = dram.tile([64, 128], bf16, name="warm_in")
            warm_out = dram.tile([8, 128], bf16, name="warm_out")
            nc.sync.dma_start(warm_in, warm_sb)
            nc.gpsimd.collective_compute(
                "ReduceScatter",
                mybir.AluOpType.add,
                replica_groups=[list(range(NCORES))],
                ins=[warm_in.opt()],
                outs=[warm_out.opt()],
            )

            # ---- collectives plumbing (separate tiles per chunk: no false deps) ----
            acc_dram = [dram.tile([RSCH, D], bf16, tag=f"accd{q}", name=f"accd{q}") for q in range(NRS)]
            rs_dram = [dram.tile([RS_ROWS, D], bf16, tag=f"rsd{q}", name=f"rsd{q}") for q in range(NRS)]

            # ---- expert FFN: chunk-outer loop; chunked overlapped RS ----
            for tch in range(NTCH):
                for tt in range(TPC):
                    do_router(tch * TPC + tt)
                for s in range(2):
                    hT = hbuf.tile([P, KF, TCH], bf16, tag="hT", name=f"hT_{tch}_{s}")
                    for fi in range(KF):
                        pg = psgu.tile([P, TCH], fp32, tag="gu", name=f"pg_{tch}_{s}_{fi}")
                        for kd in range(KD):
                            nc.tensor.matmul(
                                pg,
                                lhsT=wg_sb[s][:, kd, ts(fi, P)],
                                rhs=xTb_sb[:, kd, ts(tch, TCH)],
                                start=(kd == 0),
                                stop=(kd == KD - 1),
                            )
                        pu = psgu.tile([P, TCH], fp32, tag="gu", name=f"pu_{tch}_{s}_{fi}")
                        for kd in range(KD):
                            nc.tensor.matmul(
                                pu,
                                lhsT=wu_sb[s][:, kd, ts(fi, P)],
                                rhs=xTb_sb[:, kd, ts(tch, TCH)],
                                start=(kd == 0),
                                stop=(kd == KD - 1),
                            )
                        sil = tmp.tile([P, TCH], fp32, tag="sil")
                        nc.scalar.activation(sil, pg, ACTF.Sigmoid)
                        xg = tmp.tile([P, TCH], fp32, tag="xg")
                        nc.vector.tensor_tensor(xg, sil, pg, op=OP.mult)
                        nc.vector.tensor_tensor(hT[:, fi, :], xg, pu, op=OP.mult)
                    for tt in range(TPC):
                        g = tch * TPC + tt
                        wsel = we0_sb if s == 0 else we1_sb
                        for dn in range(D // 512):
                            py = psy.tile([P, 512], fp32, tag="y", name=f"py_{tch}_{s}_{tt}_{dn}")
                            for fi in range(KF):
                                nc.tensor.matmul(
                                    py,
                                    lhsT=hT[:, fi, ts(tt, P)],
                                    rhs=wd_sb[s][:, fi, ts(dn, 512)],
                                    start=(fi == 0),
                                    stop=(fi == KF - 1),
                                )
                            if s == 0:
                                nc.vector.tensor_scalar(
                                    acc_sb[g][:, ts(dn, 512)], py,
                                    wsel[:, g : g + 1], None, op0=OP.mult,
                                )
                            else:
                                nc.vector.scalar_tensor_tensor(
                                    acc_sb[g][:, ts(dn, 512)],
                                    in0=py,
                                    scalar=wsel[:, g : g + 1],
                                    in1=acc_sb[g][:, ts(dn, 512)],
                                    op0=OP.mult,
                                    op1=OP.add,
                                )
                # chunk complete -> bounce -> RS (tch == RS chunk index)
                for tt in range(TPC):
                    g = tch * TPC + tt
                    nc.sync.dma_start(acc_dram[tch][ts(tt, P), :], acc_sb[g])
                nc.gpsimd.collective_compute(
                    "ReduceScatter",
                    mybir.AluOpType.add,
                    replica_groups=[list(range(NCORES))],
                    ins=[acc_dram[tch].opt()],
                    outs=[rs_dram[tch].opt()],
                )
                nc.sync.dma_start(out_sh[ts(tch, RS_ROWS), :], rs_dram[tch])

    nc.compile()
    return nc


def build_in_maps(**inputs):
    import ml_dtypes

    x = np.ascontiguousarray(
        np.asarray(inputs["hidden_states"], dtype=np.float32).reshape(T, D)
    )
    w_out = np.asarray(inputs["w_out_gate"], dtype=np.float32)        # [8, D]
    w_in = np.asarray(inputs["w_in_gates"], dtype=np.float32)         # [8, 2, D]
    w_gate = np.asarray(inputs["w_gate_proj"], dtype=np.float32)      # [16, F, D]
    w_up = np.asarray(inputs["w_up_proj"], dtype=np.float32)          # [16, F, D]
    w_down = np.asarray(inputs["w_down_proj"], dtype=np.float32)      # [16, D, F]

    bf16 = ml_dtypes.bfloat16
    xT = np.ascontiguousarray(x.T)                                    # [D, T]
    wob = np.ascontiguousarray(
        np.concatenate([w_out.T, w_in.reshape(2 * NOUT, D).T], axis=1)
    ).astype(np.float32)

    in_maps = []
    for c in range(NCORES):
        e0, e1 = 2 * c, 2 * c + 1
        sm = np.zeros((1, NOUT), np.float32)
        sm[0, c] = 1.0
        in_maps.append(
            {
                "xTf": xT,
                "wg": np.ascontiguousarray(
                    np.stack([w_gate[e0].T, w_gate[e1].T])
                ).astype(bf16),
                "wu": np.ascontiguousarray(
                    np.stack([w_up[e0].T, w_up[e1].T])
                ).astype(bf16),
                "wd": np.ascontiguousarray(
                    np.stack([w_down[e0].T, w_down[e1].T])
                ).astype(bf16),
                "wob": wob,
                "selm": sm,
            }
        )
    return in_maps


def assemble_out(outs):
    """Reassemble [T, D] from per-core chunked reduce-scatter outputs."""
    out = np.empty((T, D), np.float32)
    for q in range(NRS):
        for c in range(NCORES):
            rows = np.asarray(outs[c]["out"][q * RS_ROWS : (q + 1) * RS_ROWS], dtype=np.float32)
            out[q * RSCH + c * RS_ROWS : q * RSCH + (c + 1) * RS_ROWS] = rows
    return out


def kernel(**inputs):
    global _COMPILED
    in_maps = build_in_maps(**inputs)

    if _COMPILED is None:
        _COMPILED = _build()
    nc = _COMPILED

    from concourse.bass_utils import run_bass_kernel_spmd

    res = run_bass_kernel_spmd(nc, in_maps, core_ids=list(range(NCORES)))
    outs = res.results
    out = assemble_out(outs).reshape(1, T, D).astype(np.float32)
    sel_w = outs[0]["selw"].astype(np.float32)
    return out, sel_w


# revision 23
# speedup vs baseline: 1.0902x; 1.0902x over previous
"""Compound MoE (8 outer x 2 inner experts, top2-outer / (2,1)-inner) on 8 TRN2 cores.

Strategy (expert-parallel, per sharding hint):
  - core c owns global experts {2c, 2c+1} == both inner experts of outer expert c.
  - every core computes the (cheap) router in exact fp32 on-device, then the
    dense FFN for its two experts over all tokens in bf16 (fp32 PSUM accum),
    scales by its combine weights, and ReduceScatters the [T, D] partial sums
    (chunked by 256 tokens, overlapped with compute) so core c ends up with
    rows q*32..(q+1)*32 of each 256-token chunk q. Host reassembles.
  - routing is mask-based (no integer indices needed):
      w0 = sigmoid(l_top1 - l_top2)  (== normalized top-2 softmax weight)
      slot0 (top-1 outer o1): both inner experts of o1 get weight w0
      slot1 (top-2 outer o2): argmax inner expert of o2 gets weight w1 = 1-w0
    so this core's expert weights are
      we_even = w0*mask1[c] + w1*mask2[c]*(s0>=s1)
      we_odd  = w0*mask1[c] + w1*mask2[c]*(s1>s0)
  - chunk 0's combine is deferred (y0/y1 stashed unscaled) so the first expert
    matmuls never wait on the router, whose fp32 inputs DMA later.
"""

import numpy as np

P = 128
T = 1024
D = 1024
F = 512
NOUT = 8
KD = D // P   # 8 contraction tiles
KF = F // P   # 4 f tiles
NT = T // P   # 8 token tiles
TCH = 256     # token chunk (moving free dim for g/u matmuls)
NTCH = T // TCH          # 4
TPC = TCH // P           # token tiles per chunk = 2
NCORES = 8
# uneven reduce-scatter groups: (token_start, n_tokens); fewer+bigger early ops
RS_GROUPS = [(0, 512), (512, 256), (768, 256)]

_COMPILED = None  # cached compiled graph; compile once per process


def _build():
    import concourse.bass as bass
    import concourse.mybir as mybir
    import concourse.tile as tile
    from concourse import bacc
    from concourse.bass import ts

    fp32 = mybir.dt.float32
    bf16 = mybir.dt.bfloat16
    OP = mybir.AluOpType
    ACTF = mybir.ActivationFunctionType

    nc = bacc.Bacc(
        "TRN2",
        target_bir_lowering=False,
        debug=False,
        num_devices=NCORES,
    )

    # ---- I/O ----
    xTf = nc.dram_tensor("xTf", [D, T], fp32, kind="ExternalInput").ap()
    wg = nc.dram_tensor("wg", [2, D, F], bf16, kind="ExternalInput").ap()
    wu = nc.dram_tensor("wu", [2, D, F], bf16, kind="ExternalInput").ap()
    wd = nc.dram_tensor("wd", [2, F, D], bf16, kind="ExternalInput").ap()
    wob = nc.dram_tensor("wob", [D, 3 * NOUT], fp32, kind="ExternalInput").ap()
    selm = nc.dram_tensor("selm", [1, NOUT], fp32, kind="ExternalInput").ap()
    out_sh = nc.dram_tensor("out", [P, D], bf16, kind="ExternalOutput").ap()
    selw = nc.dram_tensor("selw", [T, 3], fp32, kind="ExternalOutput").ap()

    with tile.TileContext(nc) as tc:
        with (
            tc.tile_pool(name="res", bufs=1) as res,          # resident tensors
            tc.tile_pool(name="hbuf", bufs=2) as hbuf,        # hT double buffer
            tc.tile_pool(name="tmp", bufs=2) as tmp,          # small temporaries
            tc.tile_pool(name="psgu", bufs=4, space="PSUM") as psgu,
            tc.tile_pool(name="psy", bufs=4, space="PSUM") as psy,
            tc.tile_pool(name="dram", bufs=1, space="DRAM") as dram,
        ):
            # ---- resident SBUF tensors; DMAs split/ordered for early start ----
            # ship only fp32 x^T; derive bf16 copy on DVE as chunks land
            xTf_r = xTf.rearrange("(o p) t -> p o t", p=P)
            xTf_sb = res.tile([P, KD, T], fp32)
            xTb_sb = res.tile([P, KD, T], bf16)

            wg_sb = [res.tile([P, KD, F], bf16, tag=f"wg{s}", name=f"wg{s}") for s in range(2)]
            wu_sb = [res.tile([P, KD, F], bf16, tag=f"wu{s}", name=f"wu{s}") for s in range(2)]
            wd_sb = [res.tile([P, KF, D], bf16, tag=f"wd{s}", name=f"wd{s}") for s in range(2)]

            wgr = [wg[s].rearrange("(o p) f -> p o f", p=P) for s in range(2)]
            wur = [wu[s].rearrange("(o p) f -> p o f", p=P) for s in range(2)]
            wdr = [wd[s].rearrange("(o p) d -> p o d", p=P) for s in range(2)]

            # interleave xTf chunks (cast immediately) with expert-0 gate weights
            for kd in range(KD):
                nc.sync.dma_start(xTf_sb[:, kd, :], xTf_r[:, kd, :])
                nc.sync.dma_start(wg_sb[0][:, kd, :], wgr[0][:, kd, :])
                nc.vector.tensor_copy(xTb_sb[:, kd, :], xTf_sb[:, kd, :])
            wob_sb = res.tile([P, KD, 3 * NOUT], fp32)
            nc.sync.dma_start(wob_sb, wob.rearrange("(o p) g -> p o g", p=P))
            selm_sb = res.tile([P, NOUT], fp32)
            selm_bcast = bass.AP(
                tensor=selm.tensor, offset=selm.offset,
                ap=[[0, P], selm.ap[1]],
            )
            nc.gpsimd.dma_start(out=selm_sb, in_=selm_bcast)
            for kd in range(KD):
                nc.sync.dma_start(wu_sb[0][:, kd, :], wur[0][:, kd, :])
            for fi in range(KF):
                nc.sync.dma_start(wd_sb[0][:, fi, :], wdr[0][:, fi, :])
            for kd in range(KD):
                nc.sync.dma_start(wg_sb[1][:, kd, :], wgr[1][:, kd, :])
            for kd in range(KD):
                nc.sync.dma_start(wu_sb[1][:, kd, :], wur[1][:, kd, :])
            for fi in range(KF):
                nc.sync.dma_start(wd_sb[1][:, fi, :], wdr[1][:, fi, :])

            acc_sb = [res.tile([P, D], bf16, tag=f"acc{g}", name=f"acc{g}") for g in range(NT)]
            we0_sb = res.tile([P, NT], fp32)
            we1_sb = res.tile([P, NT], fp32)
            selw_sb = res.tile([P, NT, 3], fp32)

            # ---- router: emitted per-tile, staggered into the chunk loop ----
            def do_router(i):
                plg = psy.tile([P, 3 * NOUT], fp32, tag="y")
                for kd in range(KD):
                    nc.tensor.matmul(
                        plg,
                        lhsT=xTf_sb[:, kd, ts(i, P)],
                        rhs=wob_sb[:, kd, :],
                        start=(kd == 0),
                        stop=(kd == KD - 1),
                    )
                lg = tmp.tile([P, 3 * NOUT], fp32, tag="lg")
                nc.scalar.copy(lg, plg)

                Lg = lg[:, 0:NOUT]
                Sg = lg[:, NOUT : 3 * NOUT].rearrange("p (o i) -> p o i", i=2)

                m1 = tmp.tile([P, 1], fp32, tag="m1")
                nc.vector.tensor_reduce(m1, Lg, axis=mybir.AxisListType.X, op=OP.max)
                mask1 = tmp.tile([P, NOUT], fp32, tag="mask1")
                nc.vector.tensor_scalar(mask1, Lg, m1, None, op0=OP.is_ge)
                Lm = tmp.tile([P, NOUT], fp32, tag="Lm")
                nc.vector.scalar_tensor_tensor(
                    Lm, in0=mask1, scalar=-1.0e30, in1=Lg, op0=OP.mult, op1=OP.add
                )
                m2 = tmp.tile([P, 1], fp32, tag="m2")
                nc.vector.tensor_reduce(m2, Lm, axis=mybir.AxisListType.X, op=OP.max)
                mask2 = tmp.tile([P, NOUT], fp32, tag="mask2")
                nc.vector.tensor_scalar(mask2, Lm, m2, None, op0=OP.is_ge)

                dm = tmp.tile([P, 1], fp32, tag="dm")
                nc.vector.tensor_tensor(dm, m1, m2, op=OP.subtract)
                w0 = tmp.tile([P, 1], fp32, tag="w0")
                nc.scalar.activation(w0, dm, ACTF.Sigmoid)
                w1 = tmp.tile([P, 1], fp32, tag="w1")
                nc.scalar.activation(w1, dm, ACTF.Sigmoid, scale=-1.0)

                nc.gpsimd.tensor_copy(selw_sb[:, i, 0:2], w0.to_broadcast((P, 2)))
                nc.gpsimd.tensor_copy(selw_sb[:, i, 2:3], w1)

                dlt = tmp.tile([P, NOUT], fp32, tag="dlt")
                nc.vector.tensor_tensor(dlt, Sg[:, :, 0], Sg[:, :, 1], op=OP.subtract)
                m0 = tmp.tile([P, NOUT], fp32, tag="m0")
                nc.vector.tensor_scalar(m0, dlt, 0.0, None, op0=OP.is_ge)

                A = tmp.tile([P, NOUT], fp32, tag="A")
                nc.vector.tensor_scalar(A, mask1, w0, None, op0=OP.mult)
                Bm = tmp.tile([P, NOUT], fp32, tag="Bm")
                nc.vector.tensor_scalar(Bm, mask2, w1, None, op0=OP.mult)
                t1 = tmp.tile([P, NOUT], fp32, tag="t1")
                nc.vector.tensor_tensor(t1, Bm, m0, op=OP.mult)
                ce = tmp.tile([P, NOUT], fp32, tag="ce")
                nc.vector.tensor_tensor(ce, t1, A, op=OP.add)
                t2 = tmp.tile([P, NOUT], fp32, tag="t2")
                nc.vector.tensor_tensor(t2, Bm, t1, op=OP.subtract)
                co = tmp.tile([P, NOUT], fp32, tag="co")
                nc.vector.tensor_tensor(co, t2, A, op=OP.add)

                d0 = tmp.tile([P, NOUT], fp32, tag="d0")
                nc.vector.tensor_tensor(d0, ce, selm_sb, op=OP.mult)
                nc.vector.tensor_reduce(
                    we0_sb[:, i : i + 1], d0, axis=mybir.AxisListType.X, op=OP.add
                )
                d1 = tmp.tile([P, NOUT], fp32, tag="d1")
                nc.vector.tensor_tensor(d1, co, selm_sb, op=OP.mult)
                nc.vector.tensor_reduce(
                    we1_sb[:, i : i + 1], d1, axis=mybir.AxisListType.X, op=OP.add
                )
                nc.sync.dma_start(selw[ts(i, P), :], selw_sb[:, i, :])

            # ---- collectives plumbing (separate tiles per group: no false deps) ----
            acc_dram = [dram.tile([n, D], bf16, tag=f"accd{q}", name=f"accd{q}")
                        for q, (st, n) in enumerate(RS_GROUPS)]
            rs_dram = [dram.tile([n // NCORES, D], bf16, tag=f"rsd{q}", name=f"rsd{q}")
                       for q, (st, n) in enumerate(RS_GROUPS)]

            # ---- expert FFN: chunk-outer loop; chunked overlapped RS ----
            for tch in range(NTCH):
                for tt in range(TPC):
                    do_router(tch * TPC + tt)
                for s in range(2):
                    hT = hbuf.tile([P, KF, TCH], bf16, tag="hT", name=f"hT_{tch}_{s}")
                    for fi in range(KF):
                        pg = psgu.tile([P, TCH], fp32, tag="gu", name=f"pg_{tch}_{s}_{fi}")
                        for kd in range(KD):
                            nc.tensor.matmul(
                                pg,
                                lhsT=wg_sb[s][:, kd, ts(fi, P)],
                                rhs=xTb_sb[:, kd, ts(tch, TCH)],
                                start=(kd == 0),
                                stop=(kd == KD - 1),
                            )
                        pu = psgu.tile([P, TCH], fp32, tag="gu", name=f"pu_{tch}_{s}_{fi}")
                        for kd in range(KD):
                            nc.tensor.matmul(
                                pu,
                                lhsT=wu_sb[s][:, kd, ts(fi, P)],
                                rhs=xTb_sb[:, kd, ts(tch, TCH)],
                                start=(kd == 0),
                                stop=(kd == KD - 1),
                            )
                        sil = tmp.tile([P, TCH], fp32, tag="sil")
                        nc.scalar.activation(sil, pg, ACTF.Sigmoid)
                        xg = tmp.tile([P, TCH], fp32, tag="xg")
                        nc.vector.tensor_tensor(xg, sil, pg, op=OP.mult)
                        nc.vector.tensor_tensor(hT[:, fi, :], xg, pu, op=OP.mult)
                    for tt in range(TPC):
                        g = tch * TPC + tt
                        wsel = we0_sb if s == 0 else we1_sb
                        for dn in range(D // 512):
                            py = psy.tile([P, 512], fp32, tag="y", name=f"py_{tch}_{s}_{tt}_{dn}")
                            for fi in range(KF):
                                nc.tensor.matmul(
                                    py,
                                    lhsT=hT[:, fi, ts(tt, P)],
                                    rhs=wd_sb[s][:, fi, ts(dn, 512)],
                                    start=(fi == 0),
                                    stop=(fi == KF - 1),
                                )
                            if s == 0:
                                nc.vector.tensor_scalar(
                                    acc_sb[g][:, ts(dn, 512)], py,
                                    wsel[:, g : g + 1], None, op0=OP.mult,
                                )
                            else:
                                nc.vector.scalar_tensor_tensor(
                                    acc_sb[g][:, ts(dn, 512)],
                                    in0=py,
                                    scalar=wsel[:, g : g + 1],
                                    in1=acc_sb[g][:, ts(dn, 512)],
                                    op0=OP.mult,
                                    op1=OP.add,
                                )
                # chunk complete -> bounce into its RS group; fire when group full
                for tt in range(TPC):
                    g = tch * TPC + tt
                    tok = g * P
                    for q, (st, n) in enumerate(RS_GROUPS):
                        if st <= tok < st + n:
                            nc.sync.dma_start(
                                acc_dram[q][tok - st : tok - st + P, :], acc_sb[g]
                            )
                for q, (st, n) in enumerate(RS_GROUPS):
                    if st + n == (tch + 1) * TCH:  # this chunk completes group q
                        nc.gpsimd.collective_compute(
                            "ReduceScatter",
                            mybir.AluOpType.add,
                            replica_groups=[list(range(NCORES))],
                            ins=[acc_dram[q].opt()],
                            outs=[rs_dram[q].opt()],
                        )
                        off = sum(m // NCORES for _, m in RS_GROUPS[:q])
                        nc.sync.dma_start(
                            out_sh[off : off + n // NCORES, :], rs_dram[q]
                        )

    nc.compile()
    return nc


def build_in_maps(**inputs):
    import ml_dtypes

    x = np.ascontiguousarray(
        np.asarray(inputs["hidden_states"], dtype=np.float32).reshape(T, D)
    )
    w_out = np.asarray(inputs["w_out_gate"], dtype=np.float32)        # [8, D]
    w_in = np.asarray(inputs["w_in_gates"], dtype=np.float32)         # [8, 2, D]
    w_gate = np.asarray(inputs["w_gate_proj"], dtype=np.float32)      # [16, F, D]
    w_up = np.asarray(inputs["w_up_proj"], dtype=np.float32)          # [16, F, D]
    w_down = np.asarray(inputs["w_down_proj"], dtype=np.float32)      # [16, D, F]

    bf16 = ml_dtypes.bfloat16
    xT = np.ascontiguousarray(x.T)                                    # [D, T]
    wob = np.ascontiguousarray(
        np.concatenate([w_out.T, w_in.reshape(2 * NOUT, D).T], axis=1)
    ).astype(np.float32)

    in_maps = []
    for c in range(NCORES):
        e0, e1 = 2 * c, 2 * c + 1
        sm = np.zeros((1, NOUT), np.float32)
        sm[0, c] = 1.0
        in_maps.append(
            {
                "xTf": xT,
                "wg": np.ascontiguousarray(
                    np.stack([w_gate[e0].T, w_gate[e1].T])
                ).astype(bf16),
                "wu": np.ascontiguousarray(
                    np.stack([w_up[e0].T, w_up[e1].T])
                ).astype(bf16),
                "wd": np.ascontiguousarray(
                    np.stack([w_down[e0].T, w_down[e1].T])
                ).astype(bf16),
                "wob": wob,
                "selm": sm,
            }
        )
    return in_maps


def assemble_out(outs):
    """Reassemble [T, D] from per-core grouped reduce-scatter outputs."""
    out = np.empty((T, D), np.float32)
    off = 0
    for st, n in RS_GROUPS:
        rows_pc = n // NCORES
        for c in range(NCORES):
            rows = np.asarray(outs[c]["out"][off : off + rows_pc], dtype=np.float32)
            out[st + c * rows_pc : st + (c + 1) * rows_pc] = rows
        off += rows_pc
    return out


def kernel(**inputs):
    global _COMPILED
    in_maps = build_in_maps(**inputs)

    if _COMPILED is None:
        _COMPILED = _build()
    nc = _COMPILED

    from concourse.bass_utils import run_bass_kernel_spmd

    res = run_bass_kernel_spmd(nc, in_maps, core_ids=list(range(NCORES)))
    outs = res.results
    out = assemble_out(outs).reshape(1, T, D).astype(np.float32)
    sel_w = outs[0]["selw"].astype(np.float32)
    return out, sel_w
